# revision 5
# baseline (speedup 1.0000x reference)
"""EnhancedDynamicChannelAttention Trainium2 kernel (bf16 pipeline).

Reference computation (B=16, S=2048, C=1024, H=8, HD=128):
    q[b,h,:]   = pref[b,h]*Wq[:,0] + bq
    k          = f @ Wk.T + bk ;  v = f @ Wv.T + bv       (per head slice)
    scores     = softmax_s(q . k)                          [B,H,S]
    ctx[b,h,:] = sum_s scores * v[b,s,h,:]                 [B,H,HD]
    out        = f + broadcast_s(ctx)

Algebraic folding (exact up to fp reassociation):
  - softmax shift invariance -> the q.bk term drops.
  - scores[b,h,s] = f[b,s,h,:] . qk[b,h,:]  with  qk = (pref*Wq+bq) @ Wk
  - sum_s attn = 1 -> ctx = Wv @ (sum_s attn*f[b,s,h,:]) + bv
  k/v never materialized; kernel is memory bound.

bf16 strategy (harness gate is rel_err < 2e-2; this lands ~5e-3):
  - host pre-casts features to bf16; device reads/writes bf16 DRAM
    (halves HBM traffic: 8.4 MB per core round trip).
  - DVE mul / residual adds run in bf16 (2x perf mode).
  - segmented reduce done as 2 bf16 fold-adds (d128->64->32, 2x mode)
    + fp32 tensor_reduce over the last 32 (tensor_reduce has no 2x).
  - PE matmuls consume bf16 E/f (1 cyc/row vs 4 for fp32).
  - qk/ctx row broadcasts done on the PE (stride-0 ones lhsT) instead
    of a DRAM round-trip.
  - 1/sumE folded into the per-head PE transpose via diag(recip) as the
    "identity"; bv folded in as a rank-1 PE accumulate.

Distribution: pure data parallel over batch, 2 batches per core, 8 cores.
"""

import numpy as np

B, S, C = 16, 2048, 1024
H, HD = 8, 128
N_CORES = 8
BPC = B // N_CORES          # batches per core
ST = 4                      # s-rows per partition in a super tile
P = 128
SUP = S // (P * ST)         # super tiles per batch (4)
NT = S // P                 # sub tiles per batch (16)

GP_TILES = (0,)             # super tiles whose residual add runs on GPSIMD

_CACHE = {}


def _build_program():
    import concourse.bass as bass
    import concourse.bacc as bacc
    import concourse.tile as tile
    from concourse import mybir

    f32 = mybir.dt.float32
    bf16 = mybir.dt.bfloat16

    nc = bacc.Bacc("TRN2", debug=False, num_devices=N_CORES)
    f_in = nc.dram_tensor("features", [BPC, S, C], bf16, kind="ExternalInput")
    qk_in = nc.dram_tensor("qkflat", [BPC, C], bf16, kind="ExternalInput")
    wvt_in = nc.dram_tensor("wvt", [HD, HD], f32, kind="ExternalInput")
    bvf_in = nc.dram_tensor("bvflat", [1, C], f32, kind="ExternalInput")
    id8_in = nc.dram_tensor("ident8", [8, 8], f32, kind="ExternalInput")
    ones_in = nc.dram_tensor("ones128", [P, 1], bf16, kind="ExternalInput")
    out_t = nc.dram_tensor("out", [BPC, S, C], bf16, kind="ExternalOutput")

    with tile.TileContext(nc) as tc:
        with (
            tc.tile_pool(name="fpool", bufs=BPC) as fpool,
            tc.tile_pool(name="tmppool", bufs=2) as tmppool,
            tc.tile_pool(name="spool", bufs=2 * SUP) as spool,
            tc.tile_pool(name="small", bufs=2) as small,
            tc.tile_pool(name="singles", bufs=1) as singles,
            tc.tile_pool(name="ps1", bufs=2, space="PSUM") as ps1,
            tc.tile_pool(name="ps2", bufs=2, space="PSUM") as ps2,
            tc.tile_pool(name="psbc", bufs=2, space="PSUM") as psbc,
        ):
            # ---- weights / constants ----
            ones_sb = singles.tile([P, 1], bf16)
            nc.sync.dma_start(out=ones_sb, in_=ones_in[:, :])
            qk_rows = []
            for b in range(BPC):
                qk_row = small.tile([1, C], bf16, tag="qkrow")
                nc.sync.dma_start(out=qk_row, in_=qk_in[b : b + 1, :])
                qk_rows.append(qk_row)
            # weights ride the scalar (ACT) ring so fb loads on the sync
            # ring are not queued behind their fixed DMA latency
            wvt_sb = singles.tile([HD, HD], f32)
            nc.scalar.dma_start(out=wvt_sb, in_=wvt_in[:, :])
            bvf_sb = singles.tile([1, C], f32)
            nc.scalar.dma_start(out=bvf_sb, in_=bvf_in[:, :])
            id8_sb = singles.tile([8, 8], f32)
            nc.scalar.dma_start(out=id8_sb, in_=id8_in[:, :])

            ones_lhs_bf = ones_sb[0:1, 0:1].broadcast_to([1, P])
            ones_lhs_f32 = id8_sb[0:1, 0:1].broadcast_to([1, P])

            # qk broadcast down all 128 partitions via PE rank-1 product
            qk_bcs = []
            for b in range(BPC):
                qk_ps = psbc.tile([P, C], f32, tag="bcps")
                for half in range(2):
                    sl = slice(half * 512, (half + 1) * 512)
                    nc.tensor.matmul(
                        qk_ps[:, sl], ones_lhs_bf, qk_rows[b][0:1, sl],
                        start=True, stop=True,
                    )
                qk_bc = small.tile([P, C], bf16, tag="qkbc")
                nc.scalar.copy(out=qk_bc, in_=qk_ps)
                qk_bcs.append(qk_bc)

            fbs = [None] * BPC
            ctx_bcs = [None] * BPC

            def phase_A(b):
                """loads + scores + uwf/sumE accumulation for batch b."""
                qk_bc3 = qk_bcs[b].rearrange(
                    "p (o c) -> p o c", o=1
                ).broadcast_to([P, ST, C])

                uwfSE = ps2.tile([72, 512], f32, tag="uwfSE")
                uwfA = uwfSE[0:8, :]
                uwfB = uwfSE[32:40, :]
                sumE = uwfSE[64:72, 0:1]

                fview = f_in[b].rearrange("(st p t) c -> st p t c", p=P, t=ST)

                fb = fpool.tile([P, NT, C], bf16, tag="fb")
                fbs[b] = fb
                for st in range(SUP):
                    for half in range(2):
                        lo = st * ST + half * (ST // 2)
                        nc.sync.dma_start(
                            out=fb[:, lo : lo + ST // 2, :],
                            in_=fview[st][:, half * (ST // 2) : (half + 1) * (ST // 2), :],
                        )

                    prod = tmppool.tile([P, ST, C], bf16, tag="prod")
                    nc.vector.tensor_mul(
                        prod, fb[:, st * ST : (st + 1) * ST, :], qk_bc3
                    )
                    prod4 = prod.rearrange("p t (h d) -> p t h d", h=H)
                    half1 = tmppool.tile([P, ST, H, 64], bf16, tag="half1")
                    nc.vector.tensor_add(
                        half1, prod4[:, :, :, 0:64], prod4[:, :, :, 64:128]
                    )
                    quart = tmppool.tile([P, ST, H, 32], bf16, tag="quart")
                    nc.vector.tensor_add(
                        quart, half1[:, :, :, 0:32], half1[:, :, :, 32:64]
                    )
                    scores = spool.tile([P, ST, H], f32, tag="scores")
                    nc.vector.reduce_sum(
                        scores, quart, axis=mybir.AxisListType.X
                    )
                    E_sup = spool.tile([P, ST, H], bf16, tag="esup")
                    nc.scalar.activation(
                        out=E_sup.rearrange("p t h -> p (t h)"),
                        in_=scores.rearrange("p t h -> p (t h)"),
                        func=mybir.ActivationFunctionType.Exp,
                    )

                    for t in range(ST):
                        first = st == 0 and t == 0
                        last = st == SUP - 1 and t == ST - 1
                        e_sl = E_sup[:, t, :]
                        f_sl = fb[:, st * ST + t, :]
                        nc.tensor.matmul(
                            uwfA, e_sl, f_sl[:, 0:512],
                            start=first, stop=last,
                        )
                        nc.tensor.matmul(
                            uwfB[0:8, :], e_sl, f_sl[:, 512:1024],
                            start=first, stop=last,
                        )
                        nc.tensor.matmul(
                            sumE, e_sl, ones_sb, start=first, stop=last
                        )
                return uwfA, uwfB, sumE

            def phase_B(b, uwfA, uwfB, sumE):
                """ctx tail: normalize, per-head transpose, ctx broadcast."""
                recip = small.tile([8, 1], f32, tag="recip")
                nc.vector.reciprocal(recip, sumE)

                # PSUM->SBUF copy with 1/sumE folded into the ACT
                # per-partition scale
                uwf_sb = small.tile([8, C], f32, tag="uwfsb")
                nc.scalar.mul(uwf_sb[:, 0:512], uwfA[0:8, :], recip)
                nc.scalar.mul(uwf_sb[:, 512:1024], uwfB[0:8, :], recip)

                # per-head PE transpose; column h*9 of wfT8 = wf_h / sumE_h
                wfT8_ps = ps1.tile([P, H * H], f32, tag="wft8")
                for h in range(H):
                    nc.tensor.transpose(
                        wfT8_ps[:, h * H : (h + 1) * H],
                        uwf_sb[:, h * HD : (h + 1) * HD],
                        id8_sb,
                    )
                wfT8_sb = small.tile([P, H * H], f32, tag="wft8sb")
                nc.scalar.copy(out=wfT8_sb, in_=wfT8_ps)

                # ctx row [1, C] then rank-1 PE broadcast down partitions
                ctx_row_full = psbc.tile([P, C], f32, tag="bcps")
                ctx_row_ps = ctx_row_full[0:1, :]
                for h in range(H):
                    col = h * (H + 1)
                    nc.tensor.matmul(
                        ctx_row_ps[:, h * HD : (h + 1) * HD],
                        wfT8_sb[:, col : col + 1],
                        wvt_sb,
                        start=True,
                        stop=True,
                    )
                ctx_row = small.tile([1, C], bf16, tag="ctxrowsb")
                nc.scalar.copy(out=ctx_row, in_=ctx_row_ps)

                ctx_ps = psbc.tile([P, C], f32, tag="bcps")
                for half in range(2):
                    sl = slice(half * 512, (half + 1) * 512)
                    nc.tensor.matmul(
                        ctx_ps[:, sl], ones_lhs_bf, ctx_row[0:1, sl],
                        start=True, stop=False, skip_group_check=True,
                    )
                    nc.tensor.matmul(
                        ctx_ps[:, sl], ones_lhs_f32, bvf_sb[0:1, sl],
                        start=False, stop=True, skip_group_check=True,
                    )
                ctx_bc = small.tile([P, C], bf16, tag="ctxbc")
                nc.scalar.copy(out=ctx_bc, in_=ctx_ps)
                ctx_bcs[b] = ctx_bc

            def phase_C(b):
                """residual adds + stores for batch b."""
                fb = fbs[b]
                ctx_bc = ctx_bcs[b]
                oview = out_t[b].rearrange("(st p t) c -> st p t c", p=P, t=ST)
                ctx_bc2 = ctx_bc.rearrange("p (o c) -> p o c", o=1).broadcast_to(
                    [P, ST // 2, C]
                )
                ctx_bc3 = ctx_bc.rearrange("p (o c) -> p o c", o=1).broadcast_to(
                    [P, ST, C]
                )
                for st in range(SUP):
                    if st in GP_TILES:
                        fsl = fb[:, st * ST : (st + 1) * ST, :]
                        nc.gpsimd.tensor_add(fsl, fsl, ctx_bc3)
                        nc.scalar.dma_start(out=oview[st], in_=fsl)
                    else:
                        for half in range(2):
                            lo = st * ST + half * (ST // 2)
                            osl = tmppool.tile(
                                [P, ST // 2, C], bf16, tag="ostage"
                            )
                            nc.vector.tensor_add(
                                osl, fb[:, lo : lo + ST // 2, :], ctx_bc2
                            )
                            tsl = slice(half * (ST // 2), (half + 1) * (ST // 2))
                            nc.scalar.dma_start(out=oview[st][:, tsl, :], in_=osl)

            # ---- emission order: keep DVE busy with scores back to back,
            # adds slot in afterwards ----
            acc0 = phase_A(0)
            phase_B(0, *acc0)
            acc1 = phase_A(1)
            phase_B(1, *acc1)
            phase_C(0)
            phase_C(1)

    nc.finalize()
    return nc


def _get_program():
    if "nc" not in _CACHE:
        _CACHE["nc"] = _build_program()
    return _CACHE["nc"]


def _prep_in_maps(features, preference, Wq, bq, Wk, Wv, bv):
    import ml_dtypes

    f32 = np.float32
    bf16 = ml_dtypes.bfloat16
    # qk[b,h,:] = (pref[b,h]*Wq[:,0] + bq) @ Wk   -> flat [B, C]
    q = preference[:, :, None] * Wq[:, 0][None, None, :] + bq  # [B,H,HD]
    qk = np.einsum("bhe,ed->bhd", q, Wk)  # [B,H,HD]
    qkflat = np.ascontiguousarray(qk.reshape(B, C)).astype(bf16)
    wvt = np.ascontiguousarray(Wv.T, dtype=f32)
    bvflat = np.ascontiguousarray(np.tile(bv, H)[None, :], dtype=f32)
    id8 = np.eye(8, dtype=f32)
    ones128 = np.ones([P, 1], dtype=bf16)
    fb16 = np.ascontiguousarray(features).astype(bf16)

    in_maps = []
    for i in range(N_CORES):
        sl = slice(i * BPC, (i + 1) * BPC)
        in_maps.append(
            {
                "features": fb16[sl],
                "qkflat": qkflat[sl],
                "wvt": wvt,
                "bvflat": bvflat,
                "ident8": id8,
                "ones128": ones128,
            }
        )
    return in_maps


def kernel(features, preference, Wq, bq, Wk, bk, Wv, bv, **_ignored):
    features = np.asarray(features, dtype=np.float32)
    preference = np.asarray(preference, dtype=np.float32)
    Wq = np.asarray(Wq, dtype=np.float32)
    bq = np.asarray(bq, dtype=np.float32)
    Wk = np.asarray(Wk, dtype=np.float32)
    Wv = np.asarray(Wv, dtype=np.float32)
    bv = np.asarray(bv, dtype=np.float32)

    from concourse.bass_utils import run_bass_kernel_spmd

    nc = _get_program()
    in_maps = _prep_in_maps(features, preference, Wq, bq, Wk, Wv, bv)
    res = run_bass_kernel_spmd(nc, in_maps, core_ids=list(range(N_CORES)))
    out = np.concatenate(
        [np.asarray(r["out"]).astype(np.float32) for r in res.results], axis=0
    )
    return out


# revision 7
# speedup vs baseline: 1.0867x; 1.0867x over previous
"""EnhancedDynamicChannelAttention Trainium2 kernel (bf16 pipeline).

Reference computation (B=16, S=2048, C=1024, H=8, HD=128):
    q[b,h,:]   = pref[b,h]*Wq[:,0] + bq
    k          = f @ Wk.T + bk ;  v = f @ Wv.T + bv       (per head slice)
    scores     = softmax_s(q . k)                          [B,H,S]
    ctx[b,h,:] = sum_s scores * v[b,s,h,:]                 [B,H,HD]
    out        = f + broadcast_s(ctx)

Algebraic folding (exact up to fp reassociation):
  - softmax shift invariance -> the q.bk term drops.
  - scores[b,h,s] = f[b,s,h,:] . qk[b,h,:]  with  qk = (pref*Wq+bq) @ Wk
  - sum_s attn = 1 -> ctx = Wv @ (sum_s attn*f[b,s,h,:]) + bv
  k/v never materialized; kernel is memory bound.

bf16 strategy (harness gate is rel_err < 2e-2; this lands ~5e-3):
  - host pre-casts features to bf16; device reads/writes bf16 DRAM
    (halves HBM traffic: 8.4 MB per core round trip).
  - DVE mul / residual adds run in bf16 (2x perf mode).
  - segmented reduce done as 2 bf16 fold-adds (d128->64->32, 2x mode)
    + fp32 tensor_reduce over the last 32 (tensor_reduce has no 2x).
  - PE matmuls consume bf16 E/f (1 cyc/row vs 4 for fp32).
  - uwfA/uwfB/sumE share ONE PSUM bank (partition offsets 0/32/64) so
    every PSUM pool double-buffers across batches.
  - ctx tail: per-head PE transposes (1/sumE folded into the ACT
    PSUM->SBUF copy scale), ONE ctx8 matmul via the stride-9 diagonal
    lhsT, bv8 added on DVE, then per-head selector matmuls broadcast
    ctx down all 128 partitions. No DRAM round trip, no open groups.
  - residual adds all OUT of place (in-place writes to fb create false
    tile-granular RAW hazards that stall the other adder engine).

Distribution: pure data parallel over batch, 2 batches per core, 8 cores.
"""

import numpy as np

B, S, C = 16, 2048, 1024
H, HD = 8, 128
N_CORES = 8
BPC = B // N_CORES          # batches per core
ST = 4                      # s-rows per partition in a super tile
P = 128
SUP = S // (P * ST)         # super tiles per batch (4)
NT = S // P                 # sub tiles per batch (16)

GP_TILES = (0,)             # super tiles whose residual add runs on GPSIMD

_CACHE = {}


def _build_program():
    import concourse.bass as bass
    import concourse.bacc as bacc
    import concourse.tile as tile
    from concourse import mybir

    f32 = mybir.dt.float32
    bf16 = mybir.dt.bfloat16

    nc = bacc.Bacc("TRN2", debug=False, num_devices=N_CORES)
    f_in = nc.dram_tensor("features", [BPC, S, C], bf16, kind="ExternalInput")
    qk_in = nc.dram_tensor("qkflat", [BPC, C], bf16, kind="ExternalInput")
    wvt_in = nc.dram_tensor("wvt", [HD, HD], f32, kind="ExternalInput")
    bv8_in = nc.dram_tensor("bv8", [H, HD], f32, kind="ExternalInput")
    id8_in = nc.dram_tensor("ident8", [8, 8], f32, kind="ExternalInput")
    ones_in = nc.dram_tensor("ones128", [P, 1], bf16, kind="ExternalInput")
    sel_in = nc.dram_tensor("sel8", [H, C], bf16, kind="ExternalInput")
    out_t = nc.dram_tensor("out", [BPC, S, C], bf16, kind="ExternalOutput")

    with tile.TileContext(nc) as tc:
        with (
            tc.tile_pool(name="fpool", bufs=BPC) as fpool,
            tc.tile_pool(name="tmppool", bufs=2) as tmppool,
            tc.tile_pool(name="ostages", bufs=6) as ostages,
            tc.tile_pool(name="spool", bufs=2 * SUP) as spool,
            tc.tile_pool(name="small", bufs=2) as small,
            tc.tile_pool(name="singles", bufs=1) as singles,
            tc.tile_pool(name="ps1", bufs=2, space="PSUM") as ps1,
            tc.tile_pool(name="ps2", bufs=2, space="PSUM") as ps2,
            tc.tile_pool(name="psbc", bufs=2, space="PSUM") as psbc,
        ):
            # ---- constants: everything on the scalar (ACT) ring so the
            # sync ring carries ONLY the big fb loads ----
            ones_sb = singles.tile([P, 1], bf16)
            nc.scalar.dma_start(out=ones_sb, in_=ones_in[:, :])
            qk_rows = []
            for b in range(BPC):
                qk_row = small.tile([1, C], bf16, tag="qkrow")
                nc.scalar.dma_start(out=qk_row, in_=qk_in[b : b + 1, :])
                qk_rows.append(qk_row)
            wvt_sb = singles.tile([HD, HD], f32)
            nc.scalar.dma_start(out=wvt_sb, in_=wvt_in[:, :])
            bv8_sb = singles.tile([H, HD], f32)
            nc.scalar.dma_start(out=bv8_sb, in_=bv8_in[:, :])
            id8_sb = singles.tile([8, 8], f32)
            nc.scalar.dma_start(out=id8_sb, in_=id8_in[:, :])
            sel_sb = singles.tile([H, C], bf16)
            nc.scalar.dma_start(out=sel_sb, in_=sel_in[:, :])

            ones_lhs_bf = ones_sb[0:1, 0:1].broadcast_to([1, P])

            # qk broadcast down all 128 partitions via PE rank-1 product
            qk_bcs = []
            for b in range(BPC):
                qk_ps = psbc.tile([P, C], f32, tag="bcps")
                for half in range(2):
                    sl = slice(half * 512, (half + 1) * 512)
                    nc.tensor.matmul(
                        qk_ps[:, sl], ones_lhs_bf, qk_rows[b][0:1, sl],
                        start=True, stop=True,
                    )
                qk_bc = small.tile([P, C], bf16, tag="qkbc")
                nc.scalar.copy(out=qk_bc, in_=qk_ps)
                qk_bcs.append(qk_bc)

            fbs = [None] * BPC
            ctx_bcs = [None] * BPC

            def phase_A(b):
                """loads + scores + uwf/sumE accumulation for batch b."""
                qk_bc3 = qk_bcs[b].rearrange(
                    "p (o c) -> p o c", o=1
                ).broadcast_to([P, ST, C])

                # uwfA/uwfB/sumE share one PSUM bank (offsets 0/32/64)
                uwfSE = ps2.tile([72, 512], f32, tag="uwfSE")
                uwfA = uwfSE[0:8, :]
                uwfB = uwfSE[32:40, :]
                sumE = uwfSE[64:72, 0:1]

                fview = f_in[b].rearrange("(st p t) c -> st p t c", p=P, t=ST)

                fb = fpool.tile([P, NT, C], bf16, tag="fb")
                fbs[b] = fb
                for st in range(SUP):
                    for half in range(2):
                        lo = st * ST + half * (ST // 2)
                        nc.sync.dma_start(
                            out=fb[:, lo : lo + ST // 2, :],
                            in_=fview[st][:, half * (ST // 2) : (half + 1) * (ST // 2), :],
                        )

                    prod = tmppool.tile([P, ST, C], bf16, tag="prod")
                    nc.vector.tensor_mul(
                        prod, fb[:, st * ST : (st + 1) * ST, :], qk_bc3
                    )
                    prod4 = prod.rearrange("p t (h d) -> p t h d", h=H)
                    half1 = tmppool.tile([P, ST, H, 64], bf16, tag="half1")
                    nc.vector.tensor_add(
                        half1, prod4[:, :, :, 0:64], prod4[:, :, :, 64:128]
                    )
                    quart = tmppool.tile([P, ST, H, 32], bf16, tag="quart")
                    nc.vector.tensor_add(
                        quart, half1[:, :, :, 0:32], half1[:, :, :, 32:64]
                    )
                    scores = spool.tile([P, ST, H], f32, tag="scores")
                    nc.vector.reduce_sum(
                        scores, quart, axis=mybir.AxisListType.X
                    )
                    E_sup = spool.tile([P, ST, H], bf16, tag="esup")
                    nc.scalar.activation(
                        out=E_sup.rearrange("p t h -> p (t h)"),
                        in_=scores.rearrange("p t h -> p (t h)"),
                        func=mybir.ActivationFunctionType.Exp,
                    )

                    for t in range(ST):
                        first = st == 0 and t == 0
                        last = st == SUP - 1 and t == ST - 1
                        e_sl = E_sup[:, t, :]
                        f_sl = fb[:, st * ST + t, :]
                        nc.tensor.matmul(
                            uwfA, e_sl, f_sl[:, 0:512],
                            start=first, stop=last,
                        )
                        nc.tensor.matmul(
                            uwfB, e_sl, f_sl[:, 512:1024],
                            start=first, stop=last,
                        )
                        nc.tensor.matmul(
                            sumE, e_sl, ones_sb, start=first, stop=last
                        )
                return uwfA, uwfB, sumE

            def phase_B1(b, uwfA, uwfB, sumE):
                """ctx tail part 1: normalize + transposes + ctx8 matmul."""
                recip = small.tile([8, 1], f32, tag="recip")
                nc.vector.reciprocal(recip, sumE)

                # PSUM->SBUF copy with 1/sumE folded into the ACT
                # per-partition scale
                uwf_sb = small.tile([8, C], f32, tag="uwfsb")
                nc.scalar.mul(uwf_sb[:, 0:512], uwfA, recip)
                nc.scalar.mul(uwf_sb[:, 512:1024], uwfB, recip)

                # per-head PE transpose; column h*9 of wfT8 = wf_h / sumE_h
                wft8full = ps1.tile([P, H * H + HD], f32, tag="wft8")
                wfT8_ps = wft8full[:, 0 : H * H]
                for h in range(H):
                    nc.tensor.transpose(
                        wfT8_ps[:, h * H : (h + 1) * H],
                        uwf_sb[:, h * HD : (h + 1) * HD],
                        id8_sb,
                    )
                wfT8_sb = small.tile([P, H * H], f32, tag="wft8sb")
                nc.scalar.copy(out=wfT8_sb, in_=wfT8_ps)

                # ctx8[h, :] = wf_h . WvT in ONE matmul: the stride-9
                # diagonal columns of wfT8 are the per-head wf vectors
                ctx8_ps = wft8full[0:8, H * H : H * H + HD]
                nc.tensor.matmul(
                    ctx8_ps, wfT8_sb[:, 0 : H * H : H + 1], wvt_sb,
                    start=True, stop=True,
                )
                return ctx8_ps

            def phase_B2(b, ctx8_ps):
                """ctx tail part 2: +bv, selector broadcast, bf16 copy."""
                ctx8_sb = small.tile([8, HD], bf16, tag="ctx8sb")
                nc.vector.tensor_add(ctx8_sb, ctx8_ps, bv8_sb)

                # ctx_bc[p, h*HD+d] = ctx8[h, d] for every p, via per-head
                # selector matmuls (sel[:, h*HD:..] has a 1-row at p=h)
                ctx_ps = psbc.tile([P, C], f32, tag="bcps")
                for h in range(H):
                    sl = slice(h * HD, (h + 1) * HD)
                    nc.tensor.matmul(
                        ctx_ps[:, sl], sel_sb[:, sl], ctx8_sb,
                        start=True, stop=True,
                    )
                ctx_bc = small.tile([P, C], bf16, tag="ctxbc")
                nc.scalar.copy(out=ctx_bc, in_=ctx_ps)
                ctx_bcs[b] = ctx_bc

            def phase_C(b):
                """residual adds + stores for batch b (all out of place)."""
                fb = fbs[b]
                ctx_bc = ctx_bcs[b]
                oview = out_t[b].rearrange("(st p t) c -> st p t c", p=P, t=ST)
                ctx_bc2 = ctx_bc.rearrange("p (o c) -> p o c", o=1).broadcast_to(
                    [P, ST // 2, C]
                )
                ctx_bc3 = ctx_bc.rearrange("p (o c) -> p o c", o=1).broadcast_to(
                    [P, ST, C]
                )
                for st in range(SUP):
                    if st in GP_TILES:
                        gsl = ostages.tile([P, ST, C], bf16, tag="gstage", bufs=2)
                        nc.gpsimd.tensor_add(
                            gsl, fb[:, st * ST : (st + 1) * ST, :], ctx_bc3
                        )
                        nc.scalar.dma_start(out=oview[st], in_=gsl)
                    else:
                        for half in range(2):
                            lo = st * ST + half * (ST // 2)
                            osl = ostages.tile([P, ST // 2, C], bf16, tag="ostage")
                            nc.vector.tensor_add(
                                osl, fb[:, lo : lo + ST // 2, :], ctx_bc2
                            )
                            tsl = slice(half * (ST // 2), (half + 1) * (ST // 2))
                            nc.scalar.dma_start(out=oview[st][:, tsl, :], in_=osl)

            # ---- emission order: DVE stays on scores back to back; the
            # b1 tail's DVE add is emitted after C(0) so it cannot
            # head-of-line block the b0 residual adds ----
            acc0 = phase_A(0)
            ctx8_0 = phase_B1(0, *acc0)
            phase_B2(0, ctx8_0)
            acc1 = phase_A(1)
            ctx8_1 = phase_B1(1, *acc1)
            phase_C(0)
            phase_B2(1, ctx8_1)
            phase_C(1)

    nc.finalize()
    return nc


def _get_program():
    if "nc" not in _CACHE:
        _CACHE["nc"] = _build_program()
    return _CACHE["nc"]


def _prep_in_maps(features, preference, Wq, bq, Wk, Wv, bv):
    import ml_dtypes

    f32 = np.float32
    bf16 = ml_dtypes.bfloat16
    # qk[b,h,:] = (pref[b,h]*Wq[:,0] + bq) @ Wk   -> flat [B, C]
    q = preference[:, :, None] * Wq[:, 0][None, None, :] + bq  # [B,H,HD]
    qk = np.einsum("bhe,ed->bhd", q, Wk)  # [B,H,HD]
    qkflat = np.ascontiguousarray(qk.reshape(B, C)).astype(bf16)
    wvt = np.ascontiguousarray(Wv.T, dtype=f32)
    bv8 = np.ascontiguousarray(np.broadcast_to(bv[None, :], (H, HD)), dtype=f32)
    id8 = np.eye(8, dtype=f32)
    ones128 = np.ones([P, 1], dtype=bf16)
    sel8 = np.zeros([H, C], dtype=f32)
    for h in range(H):
        sel8[h, h * HD : (h + 1) * HD] = 1.0
    sel8 = sel8.astype(bf16)
    fb16 = np.ascontiguousarray(features).astype(bf16)

    in_maps = []
    for i in range(N_CORES):
        sl = slice(i * BPC, (i + 1) * BPC)
        in_maps.append(
            {
                "features": fb16[sl],
                "qkflat": qkflat[sl],
                "wvt": wvt,
                "bv8": bv8,
                "ident8": id8,
                "ones128": ones128,
                "sel8": sel8,
            }
        )
    return in_maps


def kernel(features, preference, Wq, bq, Wk, bk, Wv, bv, **_ignored):
    features = np.asarray(features, dtype=np.float32)
    preference = np.asarray(preference, dtype=np.float32)
    Wq = np.asarray(Wq, dtype=np.float32)
    bq = np.asarray(bq, dtype=np.float32)
    Wk = np.asarray(Wk, dtype=np.float32)
    Wv = np.asarray(Wv, dtype=np.float32)
    bv = np.asarray(bv, dtype=np.float32)

    from concourse.bass_utils import run_bass_kernel_spmd

    nc = _get_program()
    in_maps = _prep_in_maps(features, preference, Wq, bq, Wk, Wv, bv)
    res = run_bass_kernel_spmd(nc, in_maps, core_ids=list(range(N_CORES)))
    out = np.concatenate(
        [np.asarray(r["out"]).astype(np.float32) for r in res.results], axis=0
    )
    return out


# revision 10
# speedup vs baseline: 1.1721x; 1.0786x over previous
"""EnhancedDynamicChannelAttention Trainium2 kernel (bf16 pipeline).

Reference computation (B=16, S=2048, C=1024, H=8, HD=128):
    q[b,h,:]   = pref[b,h]*Wq[:,0] + bq
    k          = f @ Wk.T + bk ;  v = f @ Wv.T + bv       (per head slice)
    scores     = softmax_s(q . k)                          [B,H,S]
    ctx[b,h,:] = sum_s scores * v[b,s,h,:]                 [B,H,HD]
    out        = f + broadcast_s(ctx)

Algebraic folding (exact up to fp reassociation):
  - softmax shift invariance -> the q.bk term drops.
  - scores[b,h,s] = f[b,s,h,:] . qk[b,h,:]  with  qk = (pref*Wq+bq) @ Wk
  - sum_s attn = 1 -> ctx = Wv @ (sum_s attn*f[b,s,h,:]) + bv
  k/v never materialized; kernel is memory bound.

bf16 strategy (harness gate is rel_err < 2e-2; this lands ~5e-3):
  - host pre-casts features to bf16; device reads/writes bf16 DRAM
    (halves HBM traffic: 8.4 MB per core round trip).
  - DVE mul / residual adds run in bf16 (2x perf mode).
  - segmented reduce done as 2 bf16 fold-adds (d128->64->32, 2x mode)
    + fp32 tensor_reduce over the last 32 (tensor_reduce has no 2x).
  - PE matmuls consume bf16 E/f (1 cyc/row vs 4 for fp32).
  - uwfA/uwfB/sumE share ONE PSUM bank (partition offsets 0/32/64) so
    every PSUM pool double-buffers across batches.
  - ctx tail: per-head PE transposes (1/sumE folded into the ACT
    PSUM->SBUF copy scale), ONE ctx8 matmul via the stride-9 diagonal
    lhsT, bv8 added on DVE, then per-head selector matmuls broadcast
    ctx down all 128 partitions. No DRAM round trip, no open groups.
  - residual adds all OUT of place (in-place writes to fb create false
    tile-granular RAW hazards that stall the other adder engine).

Distribution: pure data parallel over batch, 2 batches per core, 8 cores.
"""

import numpy as np

B, S, C = 16, 2048, 1024
H, HD = 8, 128
N_CORES = 8
BPC = B // N_CORES          # batches per core
ST = 4                      # s-rows per partition in a super tile
P = 128
SUP = S // (P * ST)         # super tiles per batch (4)
NT = S // P                 # sub tiles per batch (16)

_CACHE = {}


def _build_program():
    import concourse.bass as bass
    import concourse.bacc as bacc
    import concourse.tile as tile
    from concourse import mybir

    f32 = mybir.dt.float32
    bf16 = mybir.dt.bfloat16

    nc = bacc.Bacc("TRN2", debug=False, num_devices=N_CORES)
    f_in = nc.dram_tensor("features", [BPC, S, C], bf16, kind="ExternalInput")
    qk_in = nc.dram_tensor("qkflat", [BPC, C], bf16, kind="ExternalInput")
    wvt_in = nc.dram_tensor("wvt", [HD, HD], f32, kind="ExternalInput")
    bv8_in = nc.dram_tensor("bv8", [H, HD], f32, kind="ExternalInput")
    id8_in = nc.dram_tensor("ident8", [8, 8], f32, kind="ExternalInput")
    ones_in = nc.dram_tensor("ones128", [P, 1], bf16, kind="ExternalInput")
    out_t = nc.dram_tensor("out", [BPC, S, C], bf16, kind="ExternalOutput")

    with tile.TileContext(nc) as tc:
        with (
            tc.tile_pool(name="fpool", bufs=BPC) as fpool,
            tc.tile_pool(name="tmppool", bufs=2) as tmppool,
            tc.tile_pool(name="ostages", bufs=6) as ostages,
            tc.tile_pool(name="spool", bufs=2 * SUP) as spool,
            tc.tile_pool(name="small", bufs=2) as small,
            tc.tile_pool(name="singles", bufs=1) as singles,
            tc.tile_pool(name="ps1", bufs=2, space="PSUM") as ps1,
            tc.tile_pool(name="ps2", bufs=2, space="PSUM") as ps2,
            tc.tile_pool(name="dscratch", bufs=2, space="DRAM") as dscratch,
        ):
            # ---- constants: everything on the scalar (ACT) ring so the
            # sync ring carries ONLY the big fb loads ----
            # qk rows broadcast straight from DRAM on the (otherwise idle)
            # gpsimd SWDGE ring: no PE/ACT hops on the startup critical path
            qk_bcs = []
            for b in range(BPC):
                qk_bc = small.tile([P, C], bf16, tag="qkbc")
                nc.gpsimd.dma_start(
                    out=qk_bc, in_=qk_in[b : b + 1, :].to_broadcast([P, C])
                )
                qk_bcs.append(qk_bc)
            ones_sb = singles.tile([P, 1], bf16)
            nc.scalar.dma_start(out=ones_sb, in_=ones_in[:, :])
            wvt_sb = singles.tile([HD, HD], f32)
            nc.scalar.dma_start(out=wvt_sb, in_=wvt_in[:, :])
            bv8_sb = singles.tile([H, HD], f32)
            nc.scalar.dma_start(out=bv8_sb, in_=bv8_in[:, :])
            id8_sb = singles.tile([8, 8], f32)
            nc.scalar.dma_start(out=id8_sb, in_=id8_in[:, :])


            fbs = [None] * BPC
            ctx_bcs = [None] * BPC

            def phase_A(b, mid_hook=None):
                """loads + scores + uwf/sumE accumulation for batch b."""
                qk_bc3 = qk_bcs[b].rearrange(
                    "p (o c) -> p o c", o=1
                ).broadcast_to([P, ST, C])

                # uwfA/uwfB/sumE share one PSUM bank (offsets 0/32/64)
                uwfSE = ps2.tile([72, 512], f32, tag="uwfSE")
                uwfA = uwfSE[0:8, :]
                uwfB = uwfSE[32:40, :]
                sumE = uwfSE[64:72, 0:1]

                fview = f_in[b].rearrange("(st p t) c -> st p t c", p=P, t=ST)

                fb = fpool.tile([P, NT, C], bf16, tag="fb")
                fbs[b] = fb
                for st in range(SUP):
                    if st == 2 and mid_hook is not None:
                        mid_hook()
                    for half in range(2):
                        lo = st * ST + half * (ST // 2)
                        nc.sync.dma_start(
                            out=fb[:, lo : lo + ST // 2, :],
                            in_=fview[st][:, half * (ST // 2) : (half + 1) * (ST // 2), :],
                        )

                    prod = tmppool.tile([P, ST, C], bf16, tag="prod")
                    nc.vector.tensor_mul(
                        prod, fb[:, st * ST : (st + 1) * ST, :], qk_bc3
                    )
                    prod4 = prod.rearrange("p t (h d) -> p t h d", h=H)
                    half1 = tmppool.tile([P, ST, H, 64], bf16, tag="half1")
                    nc.vector.tensor_add(
                        half1, prod4[:, :, :, 0:64], prod4[:, :, :, 64:128]
                    )
                    quart = tmppool.tile([P, ST, H, 32], bf16, tag="quart")
                    nc.vector.tensor_add(
                        quart, half1[:, :, :, 0:32], half1[:, :, :, 32:64]
                    )
                    scores = spool.tile([P, ST, H], f32, tag="scores")
                    nc.vector.reduce_sum(
                        scores, quart, axis=mybir.AxisListType.X
                    )
                    E_sup = spool.tile([P, ST, H], bf16, tag="esup")
                    nc.scalar.activation(
                        out=E_sup.rearrange("p t h -> p (t h)"),
                        in_=scores.rearrange("p t h -> p (t h)"),
                        func=mybir.ActivationFunctionType.Exp,
                    )

                    for t in range(ST):
                        first = st == 0 and t == 0
                        last = st == SUP - 1 and t == ST - 1
                        e_sl = E_sup[:, t, :]
                        f_sl = fb[:, st * ST + t, :]
                        nc.tensor.matmul(
                            uwfA, e_sl, f_sl[:, 0:512],
                            start=first, stop=last,
                        )
                        nc.tensor.matmul(
                            uwfB, e_sl, f_sl[:, 512:1024],
                            start=first, stop=last,
                        )
                        nc.tensor.matmul(
                            sumE, e_sl, ones_sb, start=first, stop=last
                        )
                return uwfA, uwfB, sumE

            def phase_B1(b, uwfA, uwfB, sumE):
                """ctx tail part 1: normalize + transposes + ctx8 matmul."""
                recip = small.tile([8, 1], f32, tag="recip")
                nc.vector.reciprocal(recip, sumE)

                # PSUM->SBUF copy with 1/sumE folded into the ACT
                # per-partition scale
                uwf_sb = small.tile([8, C], f32, tag="uwfsb")
                nc.scalar.mul(uwf_sb[:, 0:512], uwfA, recip)
                nc.scalar.mul(uwf_sb[:, 512:1024], uwfB, recip)

                # per-head PE transpose; column h*9 of wfT8 = wf_h / sumE_h
                wft8full = ps1.tile([P, H * H + HD], f32, tag="wft8")
                wfT8_ps = wft8full[:, 0 : H * H]
                for h in range(H):
                    nc.tensor.transpose(
                        wfT8_ps[:, h * H : (h + 1) * H],
                        uwf_sb[:, h * HD : (h + 1) * HD],
                        id8_sb,
                    )
                wfT8_sb = small.tile([P, H * H], f32, tag="wft8sb")
                nc.scalar.copy(out=wfT8_sb, in_=wfT8_ps)

                # ctx8[h, :] = wf_h . WvT in ONE matmul: the stride-9
                # diagonal columns of wfT8 are the per-head wf vectors
                ctx8_ps = wft8full[0:8, H * H : H * H + HD]
                nc.tensor.matmul(
                    ctx8_ps, wfT8_sb[:, 0 : H * H : H + 1], wvt_sb,
                    start=True, stop=True,
                )
                return ctx8_ps

            def phase_B2(b, ctx8_ps):
                """ctx tail part 2: +bv, DRAM bounce broadcast."""
                ctx8_sb = small.tile([8, HD], bf16, tag="ctx8sb")
                nc.vector.tensor_add(ctx8_sb, ctx8_ps, bv8_sb)

                # flatten [8, HD] -> DRAM row, then the proven DRAM
                # broadcast load, both on the idle gpsimd ring
                ctx_dram = dscratch.tile([1, C], bf16, tag="ctxdram")
                nc.gpsimd.dma_start(
                    out=ctx_dram.rearrange("o (h d) -> (o h) d", h=H),
                    in_=ctx8_sb,
                )
                ctx_bc = small.tile([P, C], bf16, tag="ctxbc")
                nc.gpsimd.dma_start(
                    out=ctx_bc, in_=ctx_dram[0:1, :].to_broadcast([P, C])
                )
                ctx_bcs[b] = ctx_bc

            def phase_C(b):
                """residual adds + stores for batch b (all out of place)."""
                fb = fbs[b]
                ctx_bc = ctx_bcs[b]
                oview = out_t[b].rearrange("(st p t) c -> st p t c", p=P, t=ST)
                ctx_bc2 = ctx_bc.rearrange("p (o c) -> p o c", o=1).broadcast_to(
                    [P, ST // 2, C]
                )
                ctx_bc3 = ctx_bc.rearrange("p (o c) -> p o c", o=1).broadcast_to(
                    [P, ST, C]
                )
                for st in range(SUP):
                    for half in range(2):
                        lo = st * ST + half * (ST // 2)
                        osl = ostages.tile([P, ST // 2, C], bf16, tag="ostage")
                        nc.vector.tensor_add(
                            osl, fb[:, lo : lo + ST // 2, :], ctx_bc2
                        )
                        tsl = slice(half * (ST // 2), (half + 1) * (ST // 2))
                        ring = nc.scalar if (st * 2 + half) % 2 == 0 else nc.sync
                        ring.dma_start(out=oview[st][:, tsl, :], in_=osl)

            # ---- emission order: DVE stays on scores back to back; tail
            # DVE ops are emitted where their deps are already ready so
            # they never head-of-line block score work ----
            acc0 = phase_A(0)
            ctx8_0 = phase_B1(0, *acc0)
            acc1 = phase_A(1, mid_hook=lambda: phase_B2(0, ctx8_0))
            ctx8_1 = phase_B1(1, *acc1)
            phase_C(0)
            phase_B2(1, ctx8_1)
            phase_C(1)

    nc.finalize()
    return nc


def _get_program():
    if "nc" not in _CACHE:
        _CACHE["nc"] = _build_program()
    return _CACHE["nc"]


def _prep_in_maps(features, preference, Wq, bq, Wk, Wv, bv):
    import ml_dtypes

    f32 = np.float32
    bf16 = ml_dtypes.bfloat16
    # qk[b,h,:] = (pref[b,h]*Wq[:,0] + bq) @ Wk   -> flat [B, C]
    q = preference[:, :, None] * Wq[:, 0][None, None, :] + bq  # [B,H,HD]
    qk = np.einsum("bhe,ed->bhd", q, Wk)  # [B,H,HD]
    qkflat = np.ascontiguousarray(qk.reshape(B, C)).astype(bf16)
    wvt = np.ascontiguousarray(Wv.T, dtype=f32)
    bv8 = np.ascontiguousarray(np.broadcast_to(bv[None, :], (H, HD)), dtype=f32)
    id8 = np.eye(8, dtype=f32)
    ones128 = np.ones([P, 1], dtype=bf16)
    fb16 = np.ascontiguousarray(features).astype(bf16)

    in_maps = []
    for i in range(N_CORES):
        sl = slice(i * BPC, (i + 1) * BPC)
        in_maps.append(
            {
                "features": fb16[sl],
                "qkflat": qkflat[sl],
                "wvt": wvt,
                "bv8": bv8,
                "ident8": id8,
                "ones128": ones128,
            }
        )
    return in_maps


def kernel(features, preference, Wq, bq, Wk, bk, Wv, bv, **_ignored):
    features = np.asarray(features, dtype=np.float32)
    preference = np.asarray(preference, dtype=np.float32)
    Wq = np.asarray(Wq, dtype=np.float32)
    bq = np.asarray(bq, dtype=np.float32)
    Wk = np.asarray(Wk, dtype=np.float32)
    Wv = np.asarray(Wv, dtype=np.float32)
    bv = np.asarray(bv, dtype=np.float32)

    from concourse.bass_utils import run_bass_kernel_spmd

    nc = _get_program()
    in_maps = _prep_in_maps(features, preference, Wq, bq, Wk, Wv, bv)
    res = run_bass_kernel_spmd(nc, in_maps, core_ids=list(range(N_CORES)))
    out = np.concatenate(
        [np.asarray(r["out"]).astype(np.float32) for r in res.results], axis=0
    )
    return out
